# revision 61
# baseline (speedup 1.0000x reference)
"""Trainium2 Bass kernel for AttnDecoderRNN single step (8 NeuronCores).

Sharding:
  - Data-parallel over batch B=32 (4 per core) for attention + LSTM.
  - Tensor-parallel (vocab-sharded) output projection: V=50257 padded to
    50264 = 8*6283 columns of W_out^T per core.
  - AllGather #1: per-core y=[h_new|context] (4,512) -> (32,512).
  - AllGather #2: per-core log-softmax stats (32,2) -> (256,2); each core
    combines global max/sumexp locally and normalizes its logits shard.

Math notes:
  - scores = (eh + ee + b_attn) @ v. The eh and b_attn terms are constant
    over L, so softmax is invariant to them: attn = softmax(enc @ (We^T v)).
  - attn = exp(s - m - ln(sum)) computed on ACT; context accumulated with a
    fused DVE tensor_tensor_reduce against partition-broadcast attn rows.
"""

import sys

sys.path.insert(0, "/opt/trn_rl_repo")

import numpy as np

H = 256
V = 50257
L = 2048
B = 32
NCORES = 8
BL = B // NCORES          # 4 batches per core
VP = 53248                # padded vocab, 8 * 6656 (all padding >= index 50257)
VL = VP // NCORES         # 6656 vocab columns per core, 13 chunks of 512
NROUND = 4                # vocab col rounds of 2048 (last round = 512)
RW = 2048                 # round width
NVC = 13                  # 512-wide matmul chunks per core
BW = 2560                 # bias pack width: rows 32*(c%3), cols 512*(c//3)

# f32r matmuls rejected by the BIR verifier unless producers round to f32r;
# attention/LSTM matmuls stay fp32 (4 cy/col), W_out projection runs in bf16.
USE_F32R = False
# stride-0 partition-broadcast APs on DVE ops (fallback: gpsimd materialize).
# Rejected at lowering ("AP partition dimension must have nonzero step"),
# so broadcasts are materialized via gpsimd.partition_broadcast.
USE_BCAST_AP = False

_NC_CACHE = None


def build_nc():
    import concourse.bass as bass
    import concourse.bacc as bacc
    import concourse.mybir as mybir
    import concourse.tile as tile
    from contextlib import ExitStack

    F32 = mybir.dt.float32
    F32R = mybir.dt.float32r
    BF16 = mybir.dt.bfloat16
    AF = mybir.ActivationFunctionType
    ALU = mybir.AluOpType
    AX = mybir.AxisListType

    def r(ap):  # matmul-dtype view
        return ap.bitcast(F32R) if USE_F32R else ap

    # Bacc (not raw Bass): its compile() runs move_matmul_waits_to_ldweights
    # and generate_event_semaphores, required to satisfy the 1-wait-per-
    # instruction hardware constraint.
    nc = bacc.Bacc(
        "TRN2", target_bir_lowering=False, debug=False, num_devices=NCORES
    )

    # ---------------- I/O ----------------
    enc_h = nc.dram_tensor("enc_h", [BL, H, L], F32, kind="ExternalInput")
    emb = nc.dram_tensor("emb", [BL, H], F32, kind="ExternalInput")
    h0 = nc.dram_tensor("h0", [BL, H], F32, kind="ExternalInput")
    c0 = nc.dram_tensor("c0", [BL, H], F32, kind="ExternalInput")
    we = nc.dram_tensor("we", [H, H], F32, kind="ExternalInput")
    vcol = nc.dram_tensor("vcol", [H, 1], F32, kind="ExternalInput")
    wihT = nc.dram_tensor("wihT", [2 * H, 4 * H], F32, kind="ExternalInput")
    whhT = nc.dram_tensor("whhT", [H, 4 * H], F32, kind="ExternalInput")
    bih = nc.dram_tensor("bih", [1, 4 * H], F32, kind="ExternalInput")
    bhh = nc.dram_tensor("bhh", [1, 4 * H], F32, kind="ExternalInput")
    woutT = nc.dram_tensor("woutT", [2 * H, VL], BF16, kind="ExternalInput")
    # b_out host-packed bf16 single rows: chunk c at partition 32*(c%3),
    # cols 512*(c//3) (matmul operand bases must be 0/32/64)
    bout = nc.dram_tensor("bout", [128, BW], BF16, kind="ExternalInput")
    ident = nc.dram_tensor("ident", [128, 128], F32, kind="ExternalInput")

    logp = nc.dram_tensor("logp", [B, VL], F32, kind="ExternalOutput")
    hN = nc.dram_tensor("hn", [BL, H], F32, kind="ExternalOutput")
    cN = nc.dram_tensor("cn", [BL, H], F32, kind="ExternalOutput")
    attnO = nc.dram_tensor("attn", [BL, L], F32, kind="ExternalOutput")

    # internal DRAM for collectives
    y_loc = nc.dram_tensor("y_loc", [BL, 2 * H], F32)
    y_all = nc.dram_tensor("y_all", [B, 2 * H], F32)
    st_loc = nc.dram_tensor("st_loc", [B, 2], F32)
    st_all = nc.dram_tensor("st_all", [NCORES * B, 2], F32)
    RG = [list(range(NCORES))]

    with tile.TileContext(nc) as tc, ExitStack() as ctx:
        const = ctx.enter_context(tc.tile_pool(name="const", bufs=1))
        work = ctx.enter_context(tc.tile_pool(name="work", bufs=1))
        encp = ctx.enter_context(tc.tile_pool(name="encp", bufs=1))
        wop = ctx.enter_context(tc.tile_pool(name="wop", bufs=10))
        psT = ctx.enter_context(tc.tile_pool(name="psT", bufs=2, space="PSUM"))
        # one shared 4-bank scratch slot: TTR out / gates / exp scratch
        psBig = ctx.enter_context(tc.tile_pool(name="psBig", bufs=1, space="PSUM"))
        # one shared 1-bank slot x2: score chunks + logits chunks
        psMM = ctx.enter_context(tc.tile_pool(name="psMM", bufs=2, space="PSUM"))

        id_sb = const.tile([128, 128], F32, tag="ident")
        nc.sync.dma_start(id_sb[:, :], ident.ap()[:, :])
        # ones for K=1 outer-product matmuls (bias adds, row broadcast);
        # full-height so lhsT can be sliced at any 32-aligned base partition
        # (matmul requires lhsT.base == rhs.base)
        ones_f = const.tile([128, 8], F32, tag="ones_f")
        nc.vector.memset(ones_f[:, :], 1.0)
        ones_b = const.tile([128, 128], BF16, tag="ones_b")
        nc.vector.memset(ones_b[:, :], 1.0)

        def pe_transpose(out_sb, in_sb, p, f):
            """out_sb (f,p) = in_sb (p,f)^T via PE; copies through PSUM."""
            tmp = psT.tile([128, 128], F32, tag="tp")
            nc.tensor.transpose(tmp[:f, :p], in_sb, id_sb[:p, :p])
            nc.vector.tensor_copy(out_sb, tmp[:f, :p])

        # ---------------- ue = We^T v ----------------
        we_sb = []
        vc_sb = []
        for kc in range(2):
            t = const.tile([128, H], F32, tag=f"we{kc}")
            nc.sync.dma_start(t[:, :], we.ap()[kc * 128 : (kc + 1) * 128, :])
            we_sb.append(t)
            t2 = const.tile([128, 1], F32, tag=f"vc{kc}")
            nc.sync.dma_start(t2[:, :], vcol.ap()[kc * 128 : (kc + 1) * 128, :])
            vc_sb.append(t2)
        ue_ps = psT.tile([1, H], F32, tag="tp")
        for kc in range(2):
            nc.tensor.matmul(
                ue_ps[:, :], r(vc_sb[kc][:, :]), r(we_sb[kc][:, :]),
                start=(kc == 0), stop=(kc == 1),
            )
        ue_row = work.tile([1, H], F32, tag="ue_row")
        nc.vector.tensor_copy(ue_row[:, :], ue_ps[:, :])
        uecol = work.tile([128, 2], F32, tag="uecol")
        for kc in range(2):
            pe_transpose(
                uecol[:, kc : kc + 1],
                ue_row[:, kc * 128 : (kc + 1) * 128],
                1, 128,
            )

        # ---------------- encoder tiles (resident) ----------------
        ench = []  # [b][hh] -> (128, L)
        for b in range(BL):
            row = []
            for hh in range(2):
                t = encp.tile([128, L], F32, tag=f"ench{b}_{hh}")
                nc.sync.dma_start(
                    t[:, :], enc_h.ap()[b, hh * 128 : (hh + 1) * 128, :]
                )
                row.append(t)
            ench.append(row)

        # ---------------- scores (PE) ----------------
        # batch b lives at partition 32*b (engine APs may only start at
        # partition 0/32/64/96); other partition rows are zeroed dummies.
        s128 = work.tile([128, L], F32, tag="s128")
        nc.vector.memset(s128[:, :], 0.0)
        for b in range(BL):
            for lc in range(L // 512):
                sl = slice(lc * 512, (lc + 1) * 512)
                sp = psMM.tile([1, 512], F32, tag="mm")
                for kc in range(2):
                    nc.tensor.matmul(
                        sp[:, :],
                        r(uecol[:, kc : kc + 1]),
                        r(ench[b][kc][:, sl]),
                        start=(kc == 0), stop=(kc == 1),
                    )
                nc.vector.tensor_copy(s128[32 * b : 32 * b + 1, sl], sp[:, :])

        # ---------------- softmax over L ----------------
        m128 = work.tile([128, 1], F32, tag="m128")
        nc.vector.reduce_max(m128[:, :], s128[:, :], axis=AX.X)
        negm = work.tile([128, 1], F32, tag="negm")
        nc.vector.tensor_scalar_mul(negm[:, :], m128[:, :], -1.0)
        sum128 = work.tile([128, 1], F32, tag="sum128")
        escr = psBig.tile([128, L], F32, tag="big")
        nc.scalar.activation(
            escr[:, :], s128[:, :], AF.Exp,
            bias=negm[:, :], accum_out=sum128[:, :],
        )
        lns = work.tile([128, 1], F32, tag="lns")
        nc.scalar.activation(lns[:, :], sum128[:, :], AF.Ln)
        b2 = work.tile([128, 1], F32, tag="b2")
        nc.vector.tensor_add(b2[:, :], m128[:, :], lns[:, :])
        nc.vector.tensor_scalar_mul(b2[:, :], b2[:, :], -1.0)
        attn128 = work.tile([128, L], F32, tag="attn128")
        nc.scalar.activation(attn128[:, :], s128[:, :], AF.Exp, bias=b2[:, :])
        for b in range(BL):
            nc.sync.dma_start(
                attnO.ap()[b : b + 1, :], attn128[32 * b : 32 * b + 1, :]
            )

        # ---------------- context (DVE fused mul-reduce) ----------------
        # Broadcast attn row b to 128 partitions via ones (1,128) ⊗ row
        # outer-product matmul into PSUM (bf16 inputs, fp32 accumulate).
        attn_bf = work.tile([128, L], BF16, tag="attn_bf")
        nc.vector.tensor_copy(attn_bf[:, :], attn128[:, :])
        # xT layout: (128, 16) cols = kc*4 + b, kc in 0..3; kc 0-1 emb, 2-3 ctx
        xT = work.tile([128, 4 * BL], F32, tag="xT")
        for b in range(BL):
            # matmul operands can't sit at partition 96; DMA the bf16 attn
            # row down to a base-0 row first (DMA has no base restriction)
            row0 = work.tile([1, L], BF16, tag="row0")
            nc.sync.dma_start(row0[:, :], attn_bf[32 * b : 32 * b + 1, :])
            pb_ps = psBig.tile([128, L], F32, tag="big")
            for lc in range(L // 512):
                sl = slice(lc * 512, (lc + 1) * 512)
                nc.tensor.matmul(
                    pb_ps[:, sl],
                    ones_b[0:1, :],
                    row0[:, sl],
                    start=True, stop=True,
                )
            for hh in range(2):
                # chunk by 512 so each TTR's PSUM read stays in one bank
                ctx4 = work.tile([128, 4], F32, tag="ctx4")
                for lc in range(L // 512):
                    sl = slice(lc * 512, (lc + 1) * 512)
                    scr = work.tile([128, 512], F32, tag="bcast128")
                    # tensor_tensor_reduce crashes TRN2 hw; the custom-DVE
                    # affine_mul_reduce computes the same fused mul+row-sum
                    nc.vector.affine_mul_reduce(
                        out=scr[:, :],
                        accum_out=ctx4[:, lc : lc + 1],
                        in0=ench[b][hh][:, sl],
                        in1=pb_ps[:, sl],
                        scale=1.0,
                        bias=0.0,
                    )
                col = (2 + hh) * BL + b
                nc.vector.reduce_sum(
                    xT[:, col : col + 1], ctx4[:, :], axis=AX.X
                )

        # ---------------- LSTM ----------------
        emb_sb = work.tile([BL, H], F32, tag="emb_sb")
        nc.sync.dma_start(emb_sb[:, :], emb.ap()[:, :])
        h0_sb = work.tile([BL, H], F32, tag="h0_sb")
        nc.sync.dma_start(h0_sb[:, :], h0.ap()[:, :])
        c0_sb = work.tile([BL, H], F32, tag="c0_sb")
        nc.sync.dma_start(c0_sb[:, :], c0.ap()[:, :])

        hT = work.tile([128, 2 * BL], F32, tag="hT")
        for kc in range(2):
            sl = slice(kc * 128, (kc + 1) * 128)
            pe_transpose(xT[:, kc * BL : (kc + 1) * BL], emb_sb[:, sl], BL, 128)
            pe_transpose(hT[:, kc * BL : (kc + 1) * BL], h0_sb[:, sl], BL, 128)

        wih_sb = []
        for kc in range(4):
            t = const.tile([128, 4 * H], F32, tag=f"wih{kc}")
            nc.sync.dma_start(t[:, :], wihT.ap()[kc * 128 : (kc + 1) * 128, :])
            wih_sb.append(t)
        whh_sb = []
        for kc in range(2):
            t = const.tile([128, 4 * H], F32, tag=f"whh{kc}")
            nc.sync.dma_start(t[:, :], whhT.ap()[kc * 128 : (kc + 1) * 128, :])
            whh_sb.append(t)
        bihr = work.tile([1, 4 * H], F32, tag="bihr")
        nc.sync.dma_start(bihr[:, :], bih.ap()[:, :])
        bhhr = work.tile([1, 4 * H], F32, tag="bhhr")
        nc.sync.dma_start(bhhr[:, :], bhh.ap()[:, :])
        nc.vector.tensor_add(bihr[:, :], bihr[:, :], bhhr[:, :])

        # gates = xT.T@Wih + hT.T@Whh + ones(4)⊗(b_ih+b_hh), all in one
        # PSUM accumulation group
        g_ps = psBig.tile([BL, 4 * H], F32, tag="big")
        for nn in range(2):
            sl = slice(nn * 512, (nn + 1) * 512)
            for kc in range(4):
                nc.tensor.matmul(
                    g_ps[:, sl],
                    r(xT[:, kc * BL : (kc + 1) * BL]),
                    r(wih_sb[kc][:, sl]),
                    start=(kc == 0), stop=False,
                )
            for kc in range(2):
                nc.tensor.matmul(
                    g_ps[:, sl],
                    r(hT[:, kc * BL : (kc + 1) * BL]),
                    r(whh_sb[kc][:, sl]),
                    start=False, stop=False,
                )
            nc.tensor.matmul(
                g_ps[:, sl],
                ones_f[0:1, 0:BL],
                bihr[:, sl],
                start=False, stop=True,
            )

        ig = work.tile([BL, H], F32, tag="ig")
        nc.scalar.activation(ig[:, :], g_ps[:, 0:H], AF.Sigmoid)
        fg = work.tile([BL, H], F32, tag="fg")
        nc.scalar.activation(fg[:, :], g_ps[:, H : 2 * H], AF.Sigmoid)
        gg = work.tile([BL, H], F32, tag="gg")
        nc.scalar.activation(gg[:, :], g_ps[:, 2 * H : 3 * H], AF.Tanh)
        og = work.tile([BL, H], F32, tag="og")
        nc.scalar.activation(og[:, :], g_ps[:, 3 * H : 4 * H], AF.Sigmoid)

        t1 = work.tile([BL, H], F32, tag="t1")
        nc.vector.tensor_mul(t1[:, :], fg[:, :], c0_sb[:, :])
        t2 = work.tile([BL, H], F32, tag="t2")
        nc.vector.tensor_mul(t2[:, :], ig[:, :], gg[:, :])
        cnew = work.tile([BL, H], F32, tag="cnew")
        nc.vector.tensor_add(cnew[:, :], t1[:, :], t2[:, :])
        tanhc = work.tile([BL, H], F32, tag="tanhc")
        nc.scalar.activation(tanhc[:, :], cnew[:, :], AF.Tanh)
        hnew = work.tile([BL, H], F32, tag="hnew")
        nc.vector.tensor_mul(hnew[:, :], og[:, :], tanhc[:, :])
        nc.sync.dma_start(hN.ap()[:, :], hnew[:, :])
        nc.sync.dma_start(cN.ap()[:, :], cnew[:, :])

        # ---------------- y allgather ----------------
        y_sb = work.tile([BL, 2 * H], F32, tag="y_sb")
        nc.vector.tensor_copy(y_sb[:, 0:H], hnew[:, :])
        for hh in range(2):
            pe_transpose(
                y_sb[:, H + hh * 128 : H + (hh + 1) * 128],
                xT[:, (2 + hh) * BL : (3 + hh) * BL],
                128, BL,
            )
        nc.sync.dma_start(y_loc.ap()[:, :], y_sb[:, :])
        nc.gpsimd.collective_compute(
            "AllGather",
            ALU.bypass,
            replica_groups=RG,
            ins=[y_loc.ap().opt()],
            outs=[y_all.ap().opt()],
        )
        yall_sb = work.tile([B, 2 * H], F32, tag="yall_sb")
        nc.sync.dma_start(yall_sb[:, :], y_all.ap()[:, :])
        # yT in bf16: the W_out projection runs as a bf16 matmul
        yT = work.tile([128, 4 * B], BF16, tag="yT")
        for kc in range(4):
            pe_transpose(
                yT[:, kc * B : (kc + 1) * B],
                yall_sb[:, kc * 128 : (kc + 1) * 128],
                B, 128,
            )

        # ---------------- logits (vocab shard) ----------------
        # b_out host-packed bf16: chunk c's 512 values on single partition
        # row 32*(c%3) at cols 512*(c//3); added via ones(32)⊗row matmul.
        bout_bf = work.tile([128, BW], BF16, tag="bout_bf")
        nc.sync.dma_start(bout_bf[:, :], bout.ap()[:, :])

        # packed logits: vocab chunk c -> partitions 32*(c%4), cols 512*(c//4)
        logits = work.tile([128, RW], F32, tag="logits")
        nc.vector.memset(logits[:, (NROUND - 1) * 512 :], -1e30)

        for rnd in range(NROUND):
            c0v = rnd * RW
            cw = min(RW, VL - c0v)  # 2048, last round 512
            wo_t = []
            for kc in range(4):
                t = wop.tile([128, RW], BF16, tag="wo")
                nc.sync.dma_start(
                    t[:, :cw],
                    woutT.ap()[kc * 128 : (kc + 1) * 128, c0v : c0v + cw],
                )
                wo_t.append(t)
            nsub = (cw + 511) // 512
            for sub in range(nsub):
                c = rnd * 4 + sub
                nvc = min(512, cw - sub * 512)
                g = c % 4
                rr = c // 4
                lp_ps = psMM.tile([B, 512], F32, tag="mm")
                for kc in range(4):
                    nc.tensor.matmul(
                        lp_ps[:, :nvc],
                        yT[:, kc * B : (kc + 1) * B],
                        wo_t[kc][:, sub * 512 : sub * 512 + nvc],
                        start=(kc == 0), stop=False,
                    )
                bb = 32 * (c % 3)
                nc.tensor.matmul(
                    lp_ps[:, :nvc],
                    ones_b[bb : bb + 1, 0:B],
                    bout_bf[bb : bb + 1, 512 * (c // 3) :][:, :nvc],
                    start=False, stop=True,
                )
                dst = logits[32 * g : 32 * (g + 1), rr * 512 :][:, :nvc]
                nc.vector.tensor_copy(dst, lp_ps[:, :nvc])

        # ---------------- local log-softmax stats ----------------
        # DVE tensor-tensor ops need equal SBUF base partitions, so fold the
        # 4 partition groups by copying each into a column, then reducing.
        rmax = work.tile([128, 1], F32, tag="rmax")
        nc.vector.reduce_max(rmax[:, :], logits[:, :], axis=AX.X)
        gm4 = work.tile([B, 4], F32, tag="gm4")
        for g in range(4):
            nc.vector.tensor_copy(
                gm4[:, g : g + 1], rmax[32 * g : 32 * (g + 1), :]
            )
        gmaxl = work.tile([B, 1], F32, tag="gmaxl")
        nc.vector.reduce_max(gmaxl[:, :], gm4[:, :], axis=AX.X)
        gmax128 = work.tile([128, 1], F32, tag="gmax128")
        for g in range(4):
            nc.vector.tensor_copy(gmax128[32 * g : 32 * (g + 1), :], gmaxl[:, :])
        neg128 = work.tile([128, 1], F32, tag="neg128")
        nc.vector.tensor_scalar_mul(neg128[:, :], gmax128[:, :], -1.0)
        sl4 = work.tile([128, 4], F32, tag="sl4")
        for rr in range(4):
            scrE = psBig.tile([128, 512], F32, tag="big")
            nc.scalar.activation(
                scrE[:, :], logits[:, rr * 512 : (rr + 1) * 512], AF.Exp,
                bias=neg128[:, :], accum_out=sl4[:, rr : rr + 1],
            )
        sl128 = work.tile([128, 1], F32, tag="sl128")
        nc.vector.reduce_sum(sl128[:, :], sl4[:, :], axis=AX.X)
        sg4 = work.tile([B, 4], F32, tag="sg4")
        for g in range(4):
            nc.vector.tensor_copy(
                sg4[:, g : g + 1], sl128[32 * g : 32 * (g + 1), :]
            )
        sloc = work.tile([B, 1], F32, tag="sloc")
        nc.vector.reduce_sum(sloc[:, :], sg4[:, :], axis=AX.X)

        st_sb = work.tile([B, 2], F32, tag="st_sb")
        nc.vector.tensor_copy(st_sb[:, 0:1], gmaxl[:, :])
        nc.vector.tensor_copy(st_sb[:, 1:2], sloc[:, :])
        nc.sync.dma_start(st_loc.ap()[:, :], st_sb[:, :])
        nc.gpsimd.collective_compute(
            "AllGather",
            ALU.bypass,
            replica_groups=RG,
            ins=[st_loc.ap().opt()],
            outs=[st_all.ap().opt()],
        )
        # load stats as (32, 8, 2): b on partitions, cores on free
        stall = work.tile([B, 2 * NCORES], F32, tag="stall")
        nc.sync.dma_start(
            stall[:, :].rearrange("b (k c) -> b k c", c=2),
            st_all.ap().rearrange("(k b) c -> b k c", b=B),
        )
        stv = stall[:, :].rearrange("b (k c) -> b k c", c=2)
        gmax_cols = stv[:, :, 0]
        s_cols = stv[:, :, 1]
        gmaxg = work.tile([B, 1], F32, tag="gmaxg")
        nc.vector.reduce_max(gmaxg[:, :], gmax_cols, axis=AX.X)
        d8 = work.tile([B, NCORES], F32, tag="d8")
        nc.vector.tensor_scalar(
            d8[:, :], gmax_cols, gmaxg[:, :], None, op0=ALU.subtract
        )
        e8 = work.tile([B, NCORES], F32, tag="e8")
        nc.scalar.activation(e8[:, :], d8[:, :], AF.Exp)
        w8 = work.tile([B, NCORES], F32, tag="w8")
        nc.vector.tensor_mul(w8[:, :], e8[:, :], s_cols)
        sg = work.tile([B, 1], F32, tag="sg")
        nc.vector.reduce_sum(sg[:, :], w8[:, :], axis=AX.X)
        lnsg = work.tile([B, 1], F32, tag="lnsg")
        nc.scalar.activation(lnsg[:, :], sg[:, :], AF.Ln)
        off = work.tile([B, 1], F32, tag="off")
        nc.vector.tensor_add(off[:, :], gmaxg[:, :], lnsg[:, :])
        off128 = work.tile([128, 1], F32, tag="off128")
        for g in range(4):
            nc.vector.tensor_copy(off128[32 * g : 32 * (g + 1), :], off[:, :])
        nc.vector.tensor_scalar(
            logits[:, :], logits[:, :], off128[:, :], None, op0=ALU.subtract
        )

        # ---------------- write log_probs ----------------
        for c in range(NVC):
            nvc = min(512, VL - c * 512)
            g = c % 4
            rr = c // 4
            nc.sync.dma_start(
                logp.ap()[:, c * 512 : c * 512 + nvc],
                logits[32 * g : 32 * (g + 1), rr * 512 : rr * 512 + nvc],
            )

    nc.compile()
    return nc


def _get_nc():
    global _NC_CACHE
    if _NC_CACHE is None:
        _NC_CACHE = build_nc()
    return _NC_CACHE


def _prepare_in_maps(inputs):
    toks = np.asarray(inputs["tokens"]).astype(np.int64)
    W_attn = np.asarray(inputs["W_attn"], dtype=np.float32)
    We = np.ascontiguousarray(W_attn[:, H:])
    vc = np.ascontiguousarray(
        np.asarray(inputs["v"], dtype=np.float32).reshape(H, 1)
    )
    WihT = np.ascontiguousarray(np.asarray(inputs["W_ih"], np.float32).T)
    WhhT = np.ascontiguousarray(np.asarray(inputs["W_hh"], np.float32).T)
    b_ih = np.asarray(inputs["b_ih"], np.float32).reshape(1, -1)
    b_hh = np.asarray(inputs["b_hh"], np.float32).reshape(1, -1)
    import ml_dtypes

    WoutTp = np.zeros((2 * H, VP), ml_dtypes.bfloat16)
    WoutTp[:, :V] = np.asarray(inputs["W_out"], np.float32).T.astype(
        ml_dtypes.bfloat16
    )
    boutp = np.full((VP,), -1e30, np.float32)
    boutp[:V] = np.asarray(inputs["b_out"], np.float32)

    def pack_bias(bcore):
        bpk = np.zeros((128, BW), ml_dtypes.bfloat16)
        for c in range(NVC):
            bpk[32 * (c % 3), 512 * (c // 3) : 512 * (c // 3) + 512] = bcore[
                c * 512 : (c + 1) * 512
            ].astype(ml_dtypes.bfloat16)
        return bpk
    emb_all = np.asarray(inputs["emb_table"], np.float32)[toks]
    enc = np.asarray(inputs["encoder_outputs"], np.float32)
    encT = np.ascontiguousarray(enc.transpose(0, 2, 1))
    h0a = np.ascontiguousarray(np.asarray(inputs["hidden"], np.float32)[0])
    c0a = np.ascontiguousarray(np.asarray(inputs["cell"], np.float32)[0])
    ident = np.eye(128, dtype=np.float32)

    in_maps = []
    for j in range(NCORES):
        bs = slice(BL * j, BL * (j + 1))
        vs = slice(VL * j, VL * (j + 1))
        in_maps.append(
            {
                "enc_h": np.ascontiguousarray(encT[bs]),
                "emb": np.ascontiguousarray(emb_all[bs]),
                "h0": np.ascontiguousarray(h0a[bs]),
                "c0": np.ascontiguousarray(c0a[bs]),
                "we": We,
                "vcol": vc,
                "wihT": WihT,
                "whhT": WhhT,
                "bih": b_ih,
                "bhh": b_hh,
                "woutT": np.ascontiguousarray(WoutTp[:, vs]),
                "bout": pack_bias(boutp[vs]),
                "ident": ident,
            }
        )
    return in_maps


def kernel(**inputs):
    in_maps = _prepare_in_maps(inputs)

    from concourse.bass_utils import run_bass_kernel_spmd

    res = run_bass_kernel_spmd(
        _get_nc(), in_maps, core_ids=list(range(NCORES))
    ).results

    logp = np.concatenate([res[j]["logp"] for j in range(NCORES)], axis=1)
    logp = np.ascontiguousarray(logp[:, :V])
    hn = np.concatenate([res[j]["hn"] for j in range(NCORES)], axis=0)[None]
    cn = np.concatenate([res[j]["cn"] for j in range(NCORES)], axis=0)[None]
    at = np.concatenate([res[j]["attn"] for j in range(NCORES)], axis=0)[
        :, None, :
    ]
    return logp, hn, cn, at


# revision 92
# speedup vs baseline: 272.7695x; 272.7695x over previous
"""Trainium2 Bass kernel for AttnDecoderRNN single step (8 NeuronCores).

Sharding:
  - Data-parallel over batch B=32 (4 per core) for attention + LSTM.
  - Tensor-parallel (vocab-sharded) output projection: V=50257 padded to
    50264 = 8*6283 columns of W_out^T per core.
  - AllGather #1: per-core y=[h_new|context] (4,512) -> (32,512).
  - AllGather #2: per-core log-softmax stats (32,2) -> (256,2); each core
    combines global max/sumexp locally and normalizes its logits shard.

Math notes:
  - scores = (eh + ee + b_attn) @ v. The eh and b_attn terms are constant
    over L, so softmax is invariant to them: attn = softmax(enc @ (We^T v)).
  - attn = exp(s - m - ln(sum)) computed on ACT; context accumulated with a
    fused DVE tensor_tensor_reduce against partition-broadcast attn rows.
"""

import sys

sys.path.insert(0, "/opt/trn_rl_repo")

import numpy as np

H = 256
V = 50257
L = 2048
B = 32
NCORES = 8
BL = B // NCORES          # 4 batches per core
VP = 53248                # padded vocab, 8 * 6656 (all padding >= index 50257)
VL = VP // NCORES         # 6656 vocab columns per core, 13 chunks of 512
NROUND = 4                # vocab col rounds of 2048 (last round = 512)
RW = 2048                 # round width
NVC = 13                  # 512-wide matmul chunks per core
BW = 2560                 # bias pack width: rows 32*(c%3), cols 512*(c//3)

# f32r matmuls rejected by the BIR verifier unless producers round to f32r;
# LSTM matmuls stay fp32 (4 cy/col), W_out projection runs in bf16.
USE_F32R = False
# bf16 encoder stream: halves enc DMA, full-rate scores matmul, 2x DVE
# context reduce. Costs ~3e-3 absmax-relative on attn weights.
ENC_BF16 = True
# stride-0 partition-broadcast APs on DVE ops (fallback: gpsimd materialize).
# Rejected at lowering ("AP partition dimension must have nonzero step"),
# so broadcasts are materialized via gpsimd.partition_broadcast.
USE_BCAST_AP = False

_NC_CACHE = None


def build_nc(single_core=False):
    """single_core=True: timing-model variant — collectives replaced by
    equivalent-size local DMAs (their real latency is added analytically)."""
    import concourse.bass as bass
    import concourse.bacc as bacc
    import concourse.mybir as mybir
    import concourse.tile as tile
    from contextlib import ExitStack

    F32 = mybir.dt.float32
    F32R = mybir.dt.float32r
    BF16 = mybir.dt.bfloat16
    AF = mybir.ActivationFunctionType
    ALU = mybir.AluOpType
    AX = mybir.AxisListType

    def r(ap):  # matmul-dtype view
        return ap.bitcast(F32R) if USE_F32R else ap

    # Bacc (not raw Bass): its compile() runs move_matmul_waits_to_ldweights
    # and generate_event_semaphores, required to satisfy the 1-wait-per-
    # instruction hardware constraint.
    EDT = BF16 if ENC_BF16 else F32

    nc = bacc.Bacc(
        "TRN2", target_bir_lowering=False, debug=False,
        num_devices=1 if single_core else NCORES,
    )

    # ---------------- I/O ----------------
    enc_h = nc.dram_tensor("enc_h", [BL, H, L], EDT, kind="ExternalInput")
    emb = nc.dram_tensor("emb", [BL, H], F32, kind="ExternalInput")
    h0 = nc.dram_tensor("h0", [BL, H], F32, kind="ExternalInput")
    c0 = nc.dram_tensor("c0", [BL, H], F32, kind="ExternalInput")
    we = nc.dram_tensor("we", [H, H], F32, kind="ExternalInput")
    vcol = nc.dram_tensor("vcol", [H, 1], F32, kind="ExternalInput")
    wihT = nc.dram_tensor("wihT", [2 * H, 4 * H], BF16, kind="ExternalInput")
    whhT = nc.dram_tensor("whhT", [H, 4 * H], BF16, kind="ExternalInput")
    bih = nc.dram_tensor("bih", [1, 4 * H], BF16, kind="ExternalInput")
    bhh = nc.dram_tensor("bhh", [1, 4 * H], BF16, kind="ExternalInput")
    woutT = nc.dram_tensor("woutT", [2 * H, VL], BF16, kind="ExternalInput")
    # b_out host-packed bf16 single rows: chunk c at partition 32*(c%3),
    # cols 512*(c//3) (matmul operand bases must be 0/32/64)
    bout = nc.dram_tensor("bout", [128, BW], BF16, kind="ExternalInput")
    ident = nc.dram_tensor("ident", [128, 128], F32, kind="ExternalInput")

    logp = nc.dram_tensor("logp", [B, VL], F32, kind="ExternalOutput")
    hN = nc.dram_tensor("hn", [BL, H], F32, kind="ExternalOutput")
    cN = nc.dram_tensor("cn", [BL, H], F32, kind="ExternalOutput")
    attnO = nc.dram_tensor("attn", [BL, L], F32, kind="ExternalOutput")

    # internal DRAM for collectives
    y_loc = nc.dram_tensor("y_loc", [BL, 2 * H], F32)
    y_all = nc.dram_tensor("y_all", [B, 2 * H], F32)
    st_loc = nc.dram_tensor("st_loc", [B, 2], F32)
    st_all = nc.dram_tensor("st_all", [NCORES * B, 2], F32)
    RG = [list(range(NCORES))]

    with tile.TileContext(nc) as tc, ExitStack() as ctx:
        const = ctx.enter_context(tc.tile_pool(name="const", bufs=1))
        work = ctx.enter_context(tc.tile_pool(name="work", bufs=1))
        encp = ctx.enter_context(tc.tile_pool(name="encp", bufs=1))
        wop = ctx.enter_context(tc.tile_pool(name="wop", bufs=10))
        psT = ctx.enter_context(tc.tile_pool(name="psT", bufs=2, space="PSUM"))
        # one shared 4-bank scratch slot: TTR out / gates / exp scratch
        psBig = ctx.enter_context(tc.tile_pool(name="psBig", bufs=1, space="PSUM"))
        # one shared 1-bank slot x4: score chunks, bcast chunks, logits chunks
        psMM = ctx.enter_context(tc.tile_pool(name="psMM", bufs=2, space="PSUM"))

        # trigger ACT LUT set loads before ANY dma_start is traced, so the
        # ~1.3us/set table DMAs don't queue behind megabytes of enc/W_out.
        # The engine holds one set at a time: end on Exp (softmax needs it
        # first); later phases order their functions to minimize reloads.
        warm = work.tile([1, 32], F32, tag="actwarm")
        nc.vector.memset(warm[:, :], 1.0)
        for fn in (AF.Sigmoid, AF.Tanh, AF.Ln, AF.Exp):
            nc.scalar.activation(warm[:, :], warm[:, :], fn)

        id_sb = const.tile([128, 128], F32, tag="ident")
        nc.sync.dma_start(id_sb[:, :], ident.ap()[:, :])
        # ones for K=1 outer-product matmuls (bias adds, row broadcast);
        # full-height so lhsT can be sliced at any 32-aligned base partition
        # (matmul requires lhsT.base == rhs.base)
        ones_f = const.tile([128, 8], F32, tag="ones_f")
        nc.vector.memset(ones_f[:, :], 1.0)
        ones_b = const.tile([128, 128], BF16, tag="ones_b")
        nc.vector.memset(ones_b[:, :], 1.0)

        id_b = const.tile([128, 128], BF16, tag="id_b")
        nc.vector.tensor_copy(id_b[:, :], id_sb[:, :])

        def pe_transpose(out_sb, in_sb, p, f):
            """out_sb (f,p) = in_sb (p,f)^T via PE; copies through PSUM."""
            bf = in_sb.dtype == BF16
            tmp = psT.tile([128, 128], BF16 if bf else F32, tag="tp")
            nc.tensor.transpose(
                tmp[:f, :p], in_sb, (id_b if bf else id_sb)[:p, :p]
            )
            nc.vector.tensor_copy(out_sb, tmp[:f, :p])

        # ---------------- ue = We^T v ----------------
        we_sb = []
        vc_sb = []
        for kc in range(2):
            t = const.tile([128, H], F32, tag=f"we{kc}")
            nc.sync.dma_start(t[:, :], we.ap()[kc * 128 : (kc + 1) * 128, :])
            we_sb.append(t)
            t2 = const.tile([128, 1], F32, tag=f"vc{kc}")
            nc.sync.dma_start(t2[:, :], vcol.ap()[kc * 128 : (kc + 1) * 128, :])
            vc_sb.append(t2)
        ue_ps = psT.tile([1, H], F32, tag="tp")
        for kc in range(2):
            nc.tensor.matmul(
                ue_ps[:, :], r(vc_sb[kc][:, :]), r(we_sb[kc][:, :]),
                start=(kc == 0), stop=(kc == 1),
            )
        ue_row = work.tile([1, H], F32, tag="ue_row")
        nc.vector.tensor_copy(ue_row[:, :], ue_ps[:, :])
        uecol = work.tile([128, 2], EDT, tag="uecol")
        for kc in range(2):
            pe_transpose(
                uecol[:, kc : kc + 1],
                ue_row[:, kc * 128 : (kc + 1) * 128],
                1, 128,
            )

        # ---------------- encoder tiles (resident) ----------------
        ench = []  # [b][hh] -> (128, L)
        for b in range(BL):
            row = []
            for hh in range(2):
                t = encp.tile([128, L], EDT, tag=f"ench{b}_{hh}")
                nc.sync.dma_start(
                    t[:, :], enc_h.ap()[b, hh * 128 : (hh + 1) * 128, :]
                )
                row.append(t)
            ench.append(row)

        # ---------------- scores (PE) ----------------
        # batch b lives at partition 32*b (engine APs may only start at
        # partition 0/32/64/96); other partition rows are zeroed dummies.
        s128 = work.tile([128, L], F32, tag="s128")
        nc.vector.memset(s128[:, :], 0.0)
        for b in range(BL):
            for lc in range(L // 512):
                sl = slice(lc * 512, (lc + 1) * 512)
                sp = psMM.tile([1, 512], F32, tag="mm")
                for kc in range(2):
                    nc.tensor.matmul(
                        sp[:, :],
                        r(uecol[:, kc : kc + 1]),
                        r(ench[b][kc][:, sl]),
                        start=(kc == 0), stop=(kc == 1),
                    )
                nc.scalar.copy(s128[32 * b : 32 * b + 1, sl], sp[:, :])

        # ---------------- softmax over L ----------------
        # one Exp pass (unnormalized, with row-sum accum), then DVE
        # reciprocal + scale: keeps ACT on the Exp set (no Ln thrash)
        m128 = work.tile([128, 1], F32, tag="m128")
        nc.vector.reduce_max(m128[:, :], s128[:, :], axis=AX.X)
        negm = work.tile([128, 1], F32, tag="negm")
        nc.vector.tensor_scalar_mul(negm[:, :], m128[:, :], -1.0)
        sum128 = work.tile([128, 1], F32, tag="sum128")
        attn128 = work.tile([128, L], F32, tag="attn128")
        nc.scalar.activation(
            attn128[:, :], s128[:, :], AF.Exp,
            bias=negm[:, :], accum_out=sum128[:, :],
        )
        rsum = work.tile([128, 1], F32, tag="rsum")
        nc.vector.reciprocal(rsum[:, :], sum128[:, :])
        nc.vector.tensor_scalar_mul(attn128[:, :], attn128[:, :], rsum[:, :])
        for b in range(BL):
            nc.sync.dma_start(
                attnO.ap()[b : b + 1, :], attn128[32 * b : 32 * b + 1, :]
            )

        # ---------------- context (DVE fused mul-reduce) ----------------
        # Broadcast attn row b to 128 partitions via ones (1,128) ⊗ row
        # outer-product matmul into PSUM (bf16 inputs, fp32 accumulate).
        attn_bf = work.tile([128, L], BF16, tag="attn_bf")
        nc.vector.tensor_copy(attn_bf[:, :], attn128[:, :])
        # xT layout: (128, 16) cols = kc*4 + b, kc in 0..3; kc 0-1 emb, 2-3 ctx
        xT = work.tile([128, 4 * BL], BF16, tag="xT")
        for b in range(BL):
            # matmul operands can't sit at partition 96; DMA the bf16 attn
            # row down to a base-0 row first (DMA has no base restriction)
            row0 = work.tile([1, L], BF16, tag="row0")
            nc.sync.dma_start(row0[:, :], attn_bf[32 * b : 32 * b + 1, :])
            # per-512 chunk: ones x row broadcast into a 1-bank PSUM slot,
            # then fused mul+row-sum for both h halves (tensor_tensor_reduce
            # crashes TRN2 hw; custom-DVE affine_mul_reduce works)
            ctx4a = work.tile([128, 4], F32, tag="ctx4a")
            ctx4b = work.tile([128, 4], F32, tag="ctx4b")
            for lc in range(L // 512):
                sl = slice(lc * 512, (lc + 1) * 512)
                pbc = psMM.tile([128, 512], F32, tag="mm")
                nc.tensor.matmul(
                    pbc[:, :], ones_b[0:1, :], row0[:, sl],
                    start=True, stop=True,
                )
                for hh, ctx4 in ((0, ctx4a), (1, ctx4b)):
                    # the elementwise product is junk (only accum matters);
                    # bf16 out halves the DVE write traffic
                    scr = work.tile([128, 512], BF16, tag="bcast128")
                    nc.vector.affine_mul_reduce(
                        out=scr[:, :],
                        accum_out=ctx4[:, lc : lc + 1],
                        in0=ench[b][hh][:, sl],
                        in1=pbc[:, :],
                        scale=1.0,
                        bias=0.0,
                    )
            for hh, ctx4 in ((0, ctx4a), (1, ctx4b)):
                col = (2 + hh) * BL + b
                # f32 partials -> one final bf16 rounding into xT
                with nc.allow_low_precision(reason="ctx partials are f32"):
                    nc.vector.reduce_sum(
                        xT[:, col : col + 1], ctx4[:, :], axis=AX.X
                    )

        # ---------------- LSTM ----------------
        emb_sb = work.tile([BL, H], F32, tag="emb_sb")
        nc.sync.dma_start(emb_sb[:, :], emb.ap()[:, :])
        h0_sb = work.tile([BL, H], F32, tag="h0_sb")
        nc.sync.dma_start(h0_sb[:, :], h0.ap()[:, :])
        c0_sb = work.tile([BL, H], F32, tag="c0_sb")
        nc.sync.dma_start(c0_sb[:, :], c0.ap()[:, :])

        hT = work.tile([128, 2 * BL], BF16, tag="hT")
        for kc in range(2):
            sl = slice(kc * 128, (kc + 1) * 128)
            pe_transpose(xT[:, kc * BL : (kc + 1) * BL], emb_sb[:, sl], BL, 128)
            pe_transpose(hT[:, kc * BL : (kc + 1) * BL], h0_sb[:, sl], BL, 128)

        wih_sb = []
        for kc in range(4):
            t = const.tile([128, 4 * H], BF16, tag=f"wih{kc}")
            nc.sync.dma_start(t[:, :], wihT.ap()[kc * 128 : (kc + 1) * 128, :])
            wih_sb.append(t)
        whh_sb = []
        for kc in range(2):
            t = const.tile([128, 4 * H], BF16, tag=f"whh{kc}")
            nc.sync.dma_start(t[:, :], whhT.ap()[kc * 128 : (kc + 1) * 128, :])
            whh_sb.append(t)
        bihr = work.tile([1, 4 * H], BF16, tag="bihr")
        nc.sync.dma_start(bihr[:, :], bih.ap()[:, :])
        bhhr = work.tile([1, 4 * H], BF16, tag="bhhr")
        nc.sync.dma_start(bhhr[:, :], bhh.ap()[:, :])
        nc.vector.tensor_add(bihr[:, :], bihr[:, :], bhhr[:, :])

        # gates = xT.T@Wih + hT.T@Whh + ones(4)⊗(b_ih+b_hh), all in one
        # PSUM accumulation group
        g_ps = psBig.tile([BL, 4 * H], F32, tag="big")
        for nn in range(2):
            sl = slice(nn * 512, (nn + 1) * 512)
            for kc in range(4):
                nc.tensor.matmul(
                    g_ps[:, sl],
                    r(xT[:, kc * BL : (kc + 1) * BL]),
                    r(wih_sb[kc][:, sl]),
                    start=(kc == 0), stop=False,
                )
            for kc in range(2):
                nc.tensor.matmul(
                    g_ps[:, sl],
                    r(hT[:, kc * BL : (kc + 1) * BL]),
                    r(whh_sb[kc][:, sl]),
                    start=False, stop=False,
                )
            nc.tensor.matmul(
                g_ps[:, sl],
                ones_b[0:1, 0:BL],
                bihr[:, sl],
                start=False, stop=True,
            )

        # all sigmoids first, then tanhs: each function switch costs an ACT
        # table reload (~1.3us)
        ig = work.tile([BL, H], F32, tag="ig")
        nc.scalar.activation(ig[:, :], g_ps[:, 0:H], AF.Sigmoid)
        fg = work.tile([BL, H], F32, tag="fg")
        nc.scalar.activation(fg[:, :], g_ps[:, H : 2 * H], AF.Sigmoid)
        og = work.tile([BL, H], F32, tag="og")
        nc.scalar.activation(og[:, :], g_ps[:, 3 * H : 4 * H], AF.Sigmoid)
        gg = work.tile([BL, H], F32, tag="gg")
        nc.scalar.activation(gg[:, :], g_ps[:, 2 * H : 3 * H], AF.Tanh)

        t1 = work.tile([BL, H], F32, tag="t1")
        nc.vector.tensor_mul(t1[:, :], fg[:, :], c0_sb[:, :])
        t2 = work.tile([BL, H], F32, tag="t2")
        nc.vector.tensor_mul(t2[:, :], ig[:, :], gg[:, :])
        cnew = work.tile([BL, H], F32, tag="cnew")
        nc.vector.tensor_add(cnew[:, :], t1[:, :], t2[:, :])
        tanhc = work.tile([BL, H], F32, tag="tanhc")
        nc.scalar.activation(tanhc[:, :], cnew[:, :], AF.Tanh)
        hnew = work.tile([BL, H], F32, tag="hnew")
        nc.vector.tensor_mul(hnew[:, :], og[:, :], tanhc[:, :])
        nc.sync.dma_start(hN.ap()[:, :], hnew[:, :])
        nc.sync.dma_start(cN.ap()[:, :], cnew[:, :])

        # ---------------- y allgather ----------------
        y_sb = work.tile([BL, 2 * H], F32, tag="y_sb")
        nc.vector.tensor_copy(y_sb[:, 0:H], hnew[:, :])
        for hh in range(2):
            pe_transpose(
                y_sb[:, H + hh * 128 : H + (hh + 1) * 128],
                xT[:, (2 + hh) * BL : (3 + hh) * BL],
                128, BL,
            )
        nc.sync.dma_start(y_loc.ap()[:, :], y_sb[:, :])
        if single_core:
            for j in range(NCORES):
                nc.sync.dma_start(
                    y_all.ap()[BL * j : BL * (j + 1), :], y_loc.ap()[:, :]
                )
        else:
            nc.gpsimd.collective_compute(
                "AllGather",
                ALU.bypass,
                replica_groups=RG,
                ins=[y_loc.ap().opt()],
                outs=[y_all.ap().opt()],
            )
        yall_sb = work.tile([B, 2 * H], F32, tag="yall_sb")
        nc.sync.dma_start(yall_sb[:, :], y_all.ap()[:, :])
        # yT in bf16: the W_out projection runs as a bf16 matmul
        yT = work.tile([128, 4 * B], BF16, tag="yT")
        for kc in range(4):
            pe_transpose(
                yT[:, kc * B : (kc + 1) * B],
                yall_sb[:, kc * 128 : (kc + 1) * 128],
                B, 128,
            )

        # ---------------- logits (vocab shard) ----------------
        # b_out host-packed bf16: chunk c's 512 values on single partition
        # row 32*(c%3) at cols 512*(c//3); added via ones(32)⊗row matmul.
        bout_bf = work.tile([128, BW], BF16, tag="bout_bf")
        nc.sync.dma_start(bout_bf[:, :], bout.ap()[:, :])

        # packed logits: vocab chunk c -> partitions 32*(c%4), cols 512*(c//4)
        logits = work.tile([128, RW], F32, tag="logits")
        nc.vector.memset(logits[:, (NROUND - 1) * 512 :], -1e30)
        # per-chunk running maxes, folded after the loop (keeps the stats
        # chain off the critical tail)
        cm16 = work.tile([B, 16], F32, tag="cm16")
        nc.vector.memset(cm16[:, :], -1e30)

        for rnd in range(NROUND):
            c0v = rnd * RW
            cw = min(RW, VL - c0v)  # 2048, last round 512
            wo_t = []
            for kc in range(4):
                t = wop.tile([128, RW], BF16, tag="wo")
                nc.sync.dma_start(
                    t[:, :cw],
                    woutT.ap()[kc * 128 : (kc + 1) * 128, c0v : c0v + cw],
                )
                wo_t.append(t)
            nsub = (cw + 511) // 512
            for sub in range(nsub):
                c = rnd * 4 + sub
                nvc = min(512, cw - sub * 512)
                g = c % 4
                rr = c // 4
                lp_ps = psMM.tile([B, 512], F32, tag="mm")
                for kc in range(4):
                    nc.tensor.matmul(
                        lp_ps[:, :nvc],
                        yT[:, kc * B : (kc + 1) * B],
                        wo_t[kc][:, sub * 512 : sub * 512 + nvc],
                        start=(kc == 0), stop=False,
                    )
                bb = 32 * (c % 3)
                nc.tensor.matmul(
                    lp_ps[:, :nvc],
                    ones_b[bb : bb + 1, 0:B],
                    bout_bf[bb : bb + 1, 512 * (c // 3) :][:, :nvc],
                    start=False, stop=True,
                )
                dst = logits[32 * g : 32 * (g + 1), rr * 512 :][:, :nvc]
                if c % 2 == 0:
                    nc.scalar.copy(dst, lp_ps[:, :nvc])
                else:
                    nc.vector.tensor_copy(dst, lp_ps[:, :nvc])
                nc.vector.reduce_max(
                    cm16[:, c : c + 1], lp_ps[:, :nvc], axis=AX.X
                )

        # ---------------- local log-softmax stats ----------------
        gmaxl = work.tile([B, 1], F32, tag="gmaxl")
        nc.vector.reduce_max(gmaxl[:, :], cm16[:, :], axis=AX.X)
        gmax128 = work.tile([128, 1], F32, tag="gmax128")
        for g in range(4):
            nc.scalar.copy(gmax128[32 * g : 32 * (g + 1), :], gmaxl[:, :])
        neg128 = work.tile([128, 1], F32, tag="neg128")
        nc.vector.tensor_scalar_mul(neg128[:, :], gmax128[:, :], -1.0)
        # one exp pass; out goes to a dead bf16 scratch (only accum matters)
        escr2 = work.tile([128, L], BF16, tag="attn_bf")
        sl128 = work.tile([128, 1], F32, tag="sl128")
        nc.scalar.activation(
            escr2[:, :], logits[:, :], AF.Exp,
            bias=neg128[:, :], accum_out=sl128[:, :],
        )
        sg4 = work.tile([B, 4], F32, tag="sg4")
        for g in range(4):
            nc.vector.tensor_copy(
                sg4[:, g : g + 1], sl128[32 * g : 32 * (g + 1), :]
            )
        sloc = work.tile([B, 1], F32, tag="sloc")
        nc.vector.reduce_sum(sloc[:, :], sg4[:, :], axis=AX.X)

        nc.sync.dma_start(st_loc.ap()[:, 0:1], gmaxl[:, :])
        nc.sync.dma_start(st_loc.ap()[:, 1:2], sloc[:, :])
        if single_core:
            for j in range(NCORES):
                nc.sync.dma_start(
                    st_all.ap()[B * j : B * (j + 1), :], st_loc.ap()[:, :]
                )
        else:
            nc.gpsimd.collective_compute(
                "AllGather",
                ALU.bypass,
                replica_groups=RG,
                ins=[st_loc.ap().opt()],
                outs=[st_all.ap().opt()],
            )
        # load stats as (32, 8, 2): b on partitions, cores on free
        stall = work.tile([B, 2 * NCORES], F32, tag="stall")
        nc.sync.dma_start(
            stall[:, :].rearrange("b (k c) -> b k c", c=2),
            st_all.ap().rearrange("(k b) c -> b k c", b=B),
        )
        stv = stall[:, :].rearrange("b (k c) -> b k c", c=2)
        gmax_cols = stv[:, :, 0]
        s_cols = stv[:, :, 1]
        gmaxg = work.tile([B, 1], F32, tag="gmaxg")
        nc.vector.reduce_max(gmaxg[:, :], gmax_cols, axis=AX.X)
        d8 = work.tile([B, NCORES], F32, tag="d8")
        nc.vector.tensor_scalar(
            d8[:, :], gmax_cols, gmaxg[:, :], None, op0=ALU.subtract
        )
        e8 = work.tile([B, NCORES], F32, tag="e8")
        nc.scalar.activation(e8[:, :], d8[:, :], AF.Exp)
        w8 = work.tile([B, NCORES], F32, tag="w8")
        nc.vector.tensor_mul(w8[:, :], e8[:, :], s_cols)
        sg = work.tile([B, 1], F32, tag="sg")
        nc.vector.reduce_sum(sg[:, :], w8[:, :], axis=AX.X)
        lnsg = work.tile([B, 1], F32, tag="lnsg")
        nc.scalar.activation(lnsg[:, :], sg[:, :], AF.Ln)
        off = work.tile([B, 1], F32, tag="off")
        nc.vector.tensor_add(off[:, :], gmaxg[:, :], lnsg[:, :])
        off128 = work.tile([128, 1], F32, tag="off128")
        for g in range(4):
            nc.scalar.copy(off128[32 * g : 32 * (g + 1), :], off[:, :])
        # subtract + write out per 512-col block so out-DMAs pipeline with
        # the remaining subtracts
        for rr in range(4):
            bsl = slice(rr * 512, (rr + 1) * 512)
            nc.vector.tensor_scalar(
                logits[:, bsl], logits[:, bsl], off128[:, :], None,
                op0=ALU.subtract,
            )
            for c in range(rr * 4, min(rr * 4 + 4, NVC)):
                if c // 4 != rr:
                    continue
                nvc = min(512, VL - c * 512)
                g = c % 4
                nc.sync.dma_start(
                    logp.ap()[:, c * 512 : c * 512 + nvc],
                    logits[32 * g : 32 * (g + 1), rr * 512 : rr * 512 + nvc],
                )

    nc.compile()
    return nc


def _get_nc():
    global _NC_CACHE
    if _NC_CACHE is None:
        _NC_CACHE = build_nc()
    return _NC_CACHE


def _prepare_in_maps(inputs):
    toks = np.asarray(inputs["tokens"]).astype(np.int64)
    W_attn = np.asarray(inputs["W_attn"], dtype=np.float32)
    We = np.ascontiguousarray(W_attn[:, H:])
    vc = np.ascontiguousarray(
        np.asarray(inputs["v"], dtype=np.float32).reshape(H, 1)
    )
    import ml_dtypes as _mld2

    WihT = np.ascontiguousarray(
        np.asarray(inputs["W_ih"], np.float32).T
    ).astype(_mld2.bfloat16)
    WhhT = np.ascontiguousarray(
        np.asarray(inputs["W_hh"], np.float32).T
    ).astype(_mld2.bfloat16)
    b_ih = np.asarray(inputs["b_ih"], np.float32).reshape(1, -1).astype(
        _mld2.bfloat16
    )
    b_hh = np.asarray(inputs["b_hh"], np.float32).reshape(1, -1).astype(
        _mld2.bfloat16
    )
    import ml_dtypes

    WoutTp = np.zeros((2 * H, VP), ml_dtypes.bfloat16)
    WoutTp[:, :V] = np.asarray(inputs["W_out"], np.float32).T.astype(
        ml_dtypes.bfloat16
    )
    boutp = np.full((VP,), -1e30, np.float32)
    boutp[:V] = np.asarray(inputs["b_out"], np.float32)

    def pack_bias(bcore):
        bpk = np.zeros((128, BW), ml_dtypes.bfloat16)
        for c in range(NVC):
            bpk[32 * (c % 3), 512 * (c // 3) : 512 * (c // 3) + 512] = bcore[
                c * 512 : (c + 1) * 512
            ].astype(ml_dtypes.bfloat16)
        return bpk
    emb_all = np.asarray(inputs["emb_table"], np.float32)[toks]
    enc = np.asarray(inputs["encoder_outputs"], np.float32)
    encT = np.ascontiguousarray(enc.transpose(0, 2, 1))
    if ENC_BF16:
        import ml_dtypes as _mld

        encT = encT.astype(_mld.bfloat16)
    h0a = np.ascontiguousarray(np.asarray(inputs["hidden"], np.float32)[0])
    c0a = np.ascontiguousarray(np.asarray(inputs["cell"], np.float32)[0])
    ident = np.eye(128, dtype=np.float32)

    in_maps = []
    for j in range(NCORES):
        bs = slice(BL * j, BL * (j + 1))
        vs = slice(VL * j, VL * (j + 1))
        in_maps.append(
            {
                "enc_h": np.ascontiguousarray(encT[bs]),
                "emb": np.ascontiguousarray(emb_all[bs]),
                "h0": np.ascontiguousarray(h0a[bs]),
                "c0": np.ascontiguousarray(c0a[bs]),
                "we": We,
                "vcol": vc,
                "wihT": WihT,
                "whhT": WhhT,
                "bih": b_ih,
                "bhh": b_hh,
                "woutT": np.ascontiguousarray(WoutTp[:, vs]),
                "bout": pack_bias(boutp[vs]),
                "ident": ident,
            }
        )
    return in_maps


def kernel(**inputs):
    in_maps = _prepare_in_maps(inputs)

    from concourse.bass_utils import run_bass_kernel_spmd

    res = run_bass_kernel_spmd(
        _get_nc(), in_maps, core_ids=list(range(NCORES))
    ).results

    logp = np.concatenate([res[j]["logp"] for j in range(NCORES)], axis=1)
    logp = np.ascontiguousarray(logp[:, :V])
    hn = np.concatenate([res[j]["hn"] for j in range(NCORES)], axis=0)[None]
    cn = np.concatenate([res[j]["cn"] for j in range(NCORES)], axis=0)[None]
    at = np.concatenate([res[j]["attn"] for j in range(NCORES)], axis=0)[
        :, None, :
    ]
    return logp, hn, cn, at
